# revision 24
# baseline (speedup 1.0000x reference)
"""Trainium2 Bass kernel for nn_MoEBlock_64733747085415.

MoE block: 8 experts (top-2 combine) + shared expert, B*S = 4096 tokens,
D = 1024, I = 4096.

The reference computes every expert densely, but the top-2 combine zeroes 6
of 8 expert outputs per token -- the result only depends on each token's two
selected experts.  This kernel routes:

  - Host computes the gate (67 MFLOP: x @ gate_w.T, softmax, top-2) in f32.
    The minimum top2-vs-top3 logit gap for any token is >> f32 matmul
    rounding, so the selection matches the reference's bit-for-bit.
  - Each of the 8 cores owns one expert.  The host gathers that expert's
    selected tokens (feature-major, fp16) padded to capacity C and the core
    runs the expert FFN on just those tokens: h = gelu(w1 @ x^T + b1),
    y^T = w2^T.T @ h + b2.  ~C/N of the dense work.
  - Shared expert is token-sharded: core c runs the full shared FFN on
    tokens [512c, 512c+512).  Embarrassingly parallel -- no collectives.
  - Host combines in f32: out = concat(shared parts); out[idx_e] += w_e * y_e
    (per-token top-2 softmax weights applied on host).

Big matmuls run in fp16 (full PE speed; operand rel. rounding ~5e-4).
The PE stream is gapless: the shared phase runs first (needs only ~1 MB
of input before compute can start), all loads are issued on the sync queue
in first-needed order, and the gathered-token / expert-weight loads stream
during earlier compute.  Stores ride the gpsimd queue except the final
output tile, which goes per-chunk on the (by then drained) sync queue to
shorten the kernel tail.
"""

import math
import sys
import types

import numpy as np

import concourse.bass as bass
import concourse.mybir as mybir
import concourse.tile as tile
from concourse import bacc
from concourse import bass_utils

F32 = mybir.dt.float32
F16 = mybir.dt.float16

N_CORES = 8
N = 4096          # tokens
D = 1024          # model dim
I = 4096          # expert inner dim
E = 8             # experts
NS = N // N_CORES  # shared-expert tokens per core (512)
DT = D // 128     # 8 d-tiles
IT = I // 128     # 32 i-tiles
GI = 2            # i-tiles per w1 DMA group
G1 = IT // GI     # 16 w1 groups

_NC_CACHE = {}


def install_ntff_hook():
    """Register the axon NTFF profile hook that boot skips when the antenv
    stub lacks axon_hooks.  Needed only for trace=True runs."""
    if "antenv.axon_hooks" in sys.modules:
        return
    try:
        import trn_agent_boot.trn_boot as tb

        hook = tb._ntff_profile_via_ctypes("/opt/axon/libaxon_pjrt.so")
    except Exception:
        return
    mod = types.ModuleType("antenv.axon_hooks")
    mod.get_axon_ntff_profile_hook = lambda: hook
    mod.set_axon_ntff_profile_hook = lambda h: None
    sys.modules["antenv.axon_hooks"] = mod
    import antenv

    antenv.axon_hooks = mod
    bass_utils.upload_artifacts = lambda tmpdir: tmpdir


def _chunks(c):
    """Split c (multiple of 4) into <=512-wide, near-equal PSUM chunks."""
    nch = math.ceil(c / 512)
    lo = (c // nch) // 4 * 4
    n_hi = (c - nch * lo) // 4
    sizes = [lo + 4] * n_hi + [lo] * (nch - n_hi)
    out, o = [], 0
    for s in sizes:
        out.append((o, s))
        o += s
    return out


def build_nc(C, act="gelu"):
    CCH = _chunks(C)
    SCH = _chunks(NS)

    nc = bacc.Bacc(
        "TRN2", target_bir_lowering=False, debug=False, num_devices=N_CORES
    )

    # ---- kernel I/O (per-core) ----
    xg_d = nc.dram_tensor("xg", [128, DT, C], F16, kind="ExternalInput")
    xs_d = nc.dram_tensor("xs", [128, DT, NS], F16, kind="ExternalInput")
    w1t_d = nc.dram_tensor("w1t", [G1, 128, GI, DT, 128], F16, kind="ExternalInput")
    w2t_d = nc.dram_tensor("w2t", [DT, 128, IT, 128], F16, kind="ExternalInput")
    s1t_d = nc.dram_tensor("s1t", [G1, 128, GI, DT, 128], F16, kind="ExternalInput")
    s2t_d = nc.dram_tensor("s2t", [DT, 128, IT, 128], F16, kind="ExternalInput")
    # packed biases: [b1s(IT) | b2s(DT) | b1e(IT) | b2e(DT)] per partition
    bias_d = nc.dram_tensor("biases", [128, 2 * (IT + DT)], F32, kind="ExternalInput")
    ye_d = nc.dram_tensor("ye", [DT, 128, C], F32, kind="ExternalOutput")
    ys_d = nc.dram_tensor("ys", [DT, 128, NS], F32, kind="ExternalOutput")

    GELU = (
        mybir.ActivationFunctionType.Gelu
        if act == "gelu"
        else mybir.ActivationFunctionType.Identity
    )

    with tile.TileContext(nc) as tc:
        with (
            tc.tile_pool(name="const", bufs=1) as cpool,
            tc.tile_pool(name="wA", bufs=4) as wa_pool,
            tc.tile_pool(name="wB", bufs=2) as wb_pool,
            tc.tile_pool(name="hbuf", bufs=1) as h_pool,
            tc.tile_pool(name="ystg", bufs=3) as y_pool,
            tc.tile_pool(name="ps", bufs=8, space="PSUM") as ps_pool,
        ):
            # biases on the gpsimd queue (idle otherwise until stores)
            bias = cpool.tile([128, 2 * (IT + DT)], F32)
            nc.gpsimd.dma_start(bias, bias_d[:])
            b1s = bias[:, 0:IT]
            b2s = bias[:, IT : IT + DT]
            b1e = bias[:, IT + DT : 2 * IT + DT]
            b2e = bias[:, 2 * IT + DT :]

            # HAM warmup: ~5us of dummy matmuls on a memset tile while the
            # first loads stream in, so the PE clock-gate is at 8/8 (2.4GHz)
            # when the real stream starts (saves ~1.7us of cold-rate work).
            warm = cpool.tile([128, 128], F16)
            nc.any.memset(warm, 0.0)
            wps = ps_pool.tile([128, 128], F32, tag="ps", name="warmps")
            for _ in range(48):
                nc.tensor.matmul(wps, warm, warm, start=True, stop=True)

            # sync queue, in first-needed order: xs quarter 0 -> s1 g0 ->
            # remaining xs quarters (via inject) -> s1 g1.., s2, xg, w1, w2.
            # The xs quarters are SEPARATE tiles: multiple DMAs into one tile
            # defeat the dependency tracker (observed race: later-dt matmuls
            # ran before their quarter landed).
            QDT = DT // 4
            xsq = [cpool.tile([128, QDT, NS], F16, name=f"xsq{i}") for i in range(4)]
            xg = cpool.tile([128, DT, C], F16)
            nc.sync.dma_start(xsq[0], xs_d[:, 0:QDT, :])

            h = h_pool.tile([128, IT, C], F16, tag="h")
            hs = h_pool.tile([128, IT, NS], F16, tag="hs")

            def ffn(xparts, w1d, w2d, b1, b2, yd, chunks, pfx, inject):
                hh = h if pfx == "e" else hs
                ndt0 = xparts[0].shape[1]

                def xsl(dt_i, c0, cw):
                    part = xparts[dt_i // ndt0]
                    return part[:, dt_i % ndt0, c0 : c0 + cw]

                # phase 1: h = gelu(w1^T.T @ x^T + b1), feature-major
                for g in range(G1):
                    wt = wa_pool.tile(
                        [128, GI, DT, 128], F16, tag="wA", name=f"w1{pfx}{g}"
                    )
                    nc.sync.dma_start(wt, w1d[g])
                    if ("A", g) in inject:
                        inject[("A", g)]()
                    for ii in range(GI):
                        it = g * GI + ii
                        pcs = [
                            ps_pool.tile(
                                [128, cw], F32, tag="ps", name=f"pa{pfx}{it}_{ci}"
                            )
                            for ci, (c0, cw) in enumerate(chunks)
                        ]
                        for dt_i in range(DT):
                            for ci, (c0, cw) in enumerate(chunks):
                                nc.tensor.matmul(
                                    pcs[ci],
                                    wt[:, ii, dt_i, :],
                                    xsl(dt_i, c0, cw),
                                    start=(dt_i == 0),
                                    stop=(dt_i == DT - 1),
                                )
                        for ci, (c0, cw) in enumerate(chunks):
                            nc.scalar.activation(
                                hh[:, it, c0 : c0 + cw],
                                pcs[ci],
                                GELU,
                                bias=b1[:, it : it + 1],
                                scale=1.0,
                            )
                # phase 2: y = w2^T.T @ h + b2
                for ot in range(DT):
                    w2 = wb_pool.tile(
                        [128, IT, 128], F16, tag="wB", name=f"w2{pfx}{ot}"
                    )
                    nc.sync.dma_start(w2, w2d[ot])
                    if ("B", ot) in inject:
                        inject[("B", ot)]()
                    pys = [
                        ps_pool.tile(
                            [128, cw], F32, tag="ps", name=f"pb{pfx}{ot}_{ci}"
                        )
                        for ci, (c0, cw) in enumerate(chunks)
                    ]
                    for it in range(IT):
                        for ci, (c0, cw) in enumerate(chunks):
                            nc.tensor.matmul(
                                pys[ci],
                                w2[:, it, :],
                                hh[:, it, c0 : c0 + cw],
                                start=(it == 0),
                                stop=(it == IT - 1),
                            )
                    yb = y_pool.tile(
                        [128, chunks[-1][0] + chunks[-1][1]], F32,
                        tag=f"yb{pfx}", name=f"yb{pfx}{ot}",
                    )
                    last = pfx == "e" and ot == DT - 1
                    for ci, (c0, cw) in enumerate(chunks):
                        nc.vector.tensor_scalar_add(
                            yb[:, c0 : c0 + cw],
                            pys[ci],
                            b2[:, ot : ot + 1],
                        )
                        if last:
                            # kernel-tail stores: per chunk, on two drained
                            # HWDGE queues so the descriptor gens (~0.6us
                            # each) run in parallel
                            eng = nc.sync if ci % 2 == 0 else nc.scalar
                            eng.dma_start(
                                yd[ot, :, c0 : c0 + cw], yb[:, c0 : c0 + cw]
                            )
                    if not last:
                        nc.gpsimd.dma_start(yd[ot], yb)

            def load_xs_rest():
                # must be issued BEFORE group 0's matmuls (they read all
                # quarters): deps only look backward in program order --
                # CoreSim caught the late-issue variant as an uninit read
                for i in range(1, 4):
                    nc.sync.dma_start(xsq[i], xs_d[:, i * QDT : (i + 1) * QDT, :])

            ffn(
                xsq, s1t_d, s2t_d, b1s, b2s, ys_d, SCH, "s",
                inject={
                    ("A", 0): load_xs_rest,
                    ("B", 2): lambda: nc.sync.dma_start(xg, xg_d[:]),
                },
            )
            ffn([xg], w1t_d, w2t_d, b1e, b2e, ye_d, CCH, "e", inject={})

    nc.compile()
    return nc


def _get_nc(C, act="gelu"):
    if (C, act) not in _NC_CACHE:
        _NC_CACHE[(C, act)] = build_nc(C, act)
    return _NC_CACHE[(C, act)]


def _route(x, gate_w, top_k):
    """Host gate: f32 logits/softmax, stable top-k (ties -> lower index,
    matching jax.lax.top_k)."""
    logits = x @ np.asarray(gate_w, np.float32).T            # (N, E)
    m = logits.max(axis=-1, keepdims=True)
    ex = np.exp(logits - m, dtype=np.float32)
    p = ex / ex.sum(axis=-1, keepdims=True)                  # (N, E)
    idx = np.argsort(-p, axis=-1, kind="stable")[:, :top_k]  # (N, k)
    return p, idx


def _erf(x):
    """Abramowitz & Stegun 7.1.26, |err| < 1.5e-7 (dependency-free)."""
    s = np.sign(x)
    a = np.abs(x)
    t = 1.0 / (1.0 + 0.3275911 * a)
    poly = t * (
        0.254829592
        + t * (-0.284496736 + t * (1.421413741 + t * (-1.453152027 + t * 1.061405429)))
    )
    return s * (1.0 - poly * np.exp(-a * a))


def _ffn_host(xt, w1, b1, w2, b2):
    """Exact (f64) FFN for capacity-overflow tokens."""
    hpre = xt.astype(np.float64) @ w1.astype(np.float64).T + b1.astype(np.float64)
    hh = 0.5 * hpre * (1.0 + _erf(hpre / np.sqrt(2.0)))
    return hh @ w2.astype(np.float64).T + b2.astype(np.float64)


def _tile_w1(w):
    # (I_, D) -> [G1, 128(d_in), GI, DT, 128(i_in)]
    return np.ascontiguousarray(
        w.reshape(G1, GI, 128, DT, 128).transpose(0, 4, 1, 3, 2)
    ).astype(np.float16)


def _tile_w2(w):
    # (D, I_) -> [DT, 128(i_in), IT_, 128(d_in)]
    it_ = w.shape[1] // 128
    return np.ascontiguousarray(
        w.reshape(DT, 128, it_, 128).transpose(0, 3, 2, 1)
    ).astype(np.float16)


def run(inputs, trace=False, trace_cores=None):
    """Route on host, run the FFN batch on 8 cores, combine on host."""
    x = np.ascontiguousarray(
        np.asarray(inputs["hidden_states"], np.float32).reshape(N, D)
    )
    top_k = int(inputs.get("top_k", 2))
    p, idx = _route(x, inputs["gate_w"], top_k)

    # capacity-1.0 routing: device batches are capped at N // E tokens per
    # expert (perfect balance, clean 2x512 PSUM chunks); the few overflow
    # (token, expert) pairs -- lowest-weight first -- run on host in f64.
    cap = N * top_k // E
    tok_lists, ovf_lists = [], []
    for e in range(N_CORES):
        toks = np.nonzero((idx == e).any(axis=1))[0]
        if len(toks) > cap:
            order = np.argsort(p[toks, e], kind="stable")
            ovf_lists.append(toks[order[: len(toks) - cap]])
            toks = np.sort(toks[order[len(toks) - cap :]])
        else:
            ovf_lists.append(np.empty(0, np.int64))
        tok_lists.append(toks)
    cmax = max(len(t) for t in tok_lists)
    C = max(256, -(-cmax // 4) * 4)  # round up to 4
    nc = _get_nc(C)

    # feature-major tokens: (128 d_in, DT, N)
    xT16 = np.ascontiguousarray(
        x.reshape(N, DT, 128).transpose(2, 1, 0)
    ).astype(np.float16)

    b1s = np.asarray(inputs["s_b1"], np.float32).reshape(IT, 128).T
    b2s = np.asarray(inputs["s_b2"], np.float32).reshape(DT, 128).T
    s1t = _tile_w1(np.asarray(inputs["s_w1"], np.float32))
    s2t = _tile_w2(np.asarray(inputs["s_w2"], np.float32))

    in_maps = []
    for e in range(N_CORES):
        toks = tok_lists[e]
        xg = np.zeros((128, DT, C), np.float16)
        xg[:, :, : len(toks)] = xT16[:, :, toks]
        b1e = np.asarray(inputs["e_b1"][e], np.float32).reshape(IT, 128).T
        b2e = np.asarray(inputs["e_b2"][e], np.float32).reshape(DT, 128).T
        in_maps.append(
            {
                "xg": xg,
                "xs": np.ascontiguousarray(xT16[:, :, e * NS : (e + 1) * NS]),
                "w1t": _tile_w1(np.asarray(inputs["e_w1"][e], np.float32)),
                "w2t": _tile_w2(np.asarray(inputs["e_w2"][e], np.float32)),
                "s1t": s1t,
                "s2t": s2t,
                "biases": np.ascontiguousarray(
                    np.concatenate([b1s, b2s, b1e, b2e], axis=1)
                ),
            }
        )

    if trace:
        install_ntff_hook()
    res = bass_utils.run_bass_kernel_spmd(
        nc,
        in_maps,
        core_ids=list(range(N_CORES)),
        trace=trace,
        trace_cores=trace_cores,
    )

    out = np.empty((N, D), np.float32)
    for c in range(N_CORES):
        ys = res.results[c]["ys"]  # (DT, 128, NS)
        out[c * NS : (c + 1) * NS] = ys.reshape(D, NS).T
    for e in range(N_CORES):
        toks = tok_lists[e]
        ye = res.results[e]["ye"].reshape(D, C)[:, : len(toks)]  # (D, ntok)
        out[toks] += p[toks, e][:, None] * ye.T
        ovf = ovf_lists[e]
        if len(ovf):
            yh = _ffn_host(
                x[ovf],
                np.asarray(inputs["e_w1"][e], np.float32),
                np.asarray(inputs["e_b1"][e], np.float32),
                np.asarray(inputs["e_w2"][e], np.float32),
                np.asarray(inputs["e_b2"][e], np.float32),
            )
            out[ovf] += p[ovf, e][:, None] * yh.astype(np.float32)
    return out.reshape(2, N // 2, D), res


def kernel(**inputs):
    out, _ = run(inputs, trace=False)
    return out
